# revision 1
# baseline (speedup 1.0000x reference)
"""GroupedQueryAttention distributed across 8 Trainium2 NeuronCores.

Sharding (per spec hint): data-parallel over batch B=2, tensor-parallel over
the 4 KV head groups -> 8 shards, one per core. Core c handles batch b=c//4,
kv-group g=c%4 (query heads 4g..4g+3, kv head g). Wq/Wk/Wv are column-sharded,
Wo row-sharded (Megatron style); the row-parallel partial outputs are reduced
on the host (equivalent to the tensor-parallel all-reduce).
"""
import math
import numpy as np

N_HEADS = 16
N_KV_HEADS = 4
D_HEAD = 128
GROUPS = N_HEADS // N_KV_HEADS  # 4
EPS = 1e-6
THETA = 10000.0
B, S, D = 2, 2048, 2048
N_CORES = 8


def _rope_tables(dtype=np.float32):
    freqs = 1.0 / THETA ** (np.arange(0, D_HEAD, 2, dtype=np.float64) / D_HEAD)
    angles = np.arange(S, dtype=np.float64)[:, None] * freqs[None, :]
    angles = np.concatenate([angles, angles], axis=-1)  # [S, D_HEAD]
    return np.cos(angles).astype(dtype), np.sin(angles).astype(dtype)


def _shard_inputs(x, Wq, Wk, Wv, Wo, q_norm_w, k_norm_w):
    """Per-core input tuples: core c -> (b=c//4, g=c%4)."""
    shards = []
    for c in range(N_CORES):
        b, g = divmod(c, GROUPS)
        shards.append(dict(
            x=np.ascontiguousarray(x[b]),                                   # [S, D]
            wq=np.ascontiguousarray(Wq[:, g * GROUPS * D_HEAD:(g + 1) * GROUPS * D_HEAD]),  # [D, 512]
            wk=np.ascontiguousarray(Wk[:, g * D_HEAD:(g + 1) * D_HEAD]),    # [D, 128]
            wv=np.ascontiguousarray(Wv[:, g * D_HEAD:(g + 1) * D_HEAD]),    # [D, 128]
            wo=np.ascontiguousarray(Wo[g * GROUPS * D_HEAD:(g + 1) * GROUPS * D_HEAD, :]),  # [512, D]
            qw=q_norm_w, kw=k_norm_w,
        ))
    return shards


def _gqa_shard_np(x, wq, wk, wv, wo, qw, kw, cos, sin):
    """One core's share: 4 query heads + 1 kv head of causal GQA, fp32 numpy."""
    q = (x @ wq).reshape(S, GROUPS, D_HEAD).transpose(1, 0, 2)  # [4, S, d]
    k = x @ wk                                                  # [S, d]
    v = x @ wv

    def rms(t, w):
        var = np.mean(t * t, axis=-1, keepdims=True)
        return t / np.sqrt(var + EPS) * w

    q = rms(q, qw)
    k = rms(k, kw)

    def rope(t):
        half = D_HEAD // 2
        rot = np.concatenate([-t[..., half:], t[..., :half]], axis=-1)
        return t * cos + rot * sin

    q = rope(q)
    k = rope(k[None])[0]

    scale = 1.0 / math.sqrt(D_HEAD)
    out_heads = np.empty((GROUPS, S, D_HEAD), np.float32)
    neg = np.float32(np.finfo(np.float32).min)
    mask = np.triu(np.ones((S, S), bool), 1)
    for h in range(GROUPS):
        s = (q[h] @ k.T) * scale
        s[mask] = neg
        s -= s.max(axis=-1, keepdims=True)
        e = np.exp(s, dtype=np.float32)
        p = e / e.sum(axis=-1, keepdims=True)
        out_heads[h] = p @ v
    attn = out_heads.transpose(1, 0, 2).reshape(S, GROUPS * D_HEAD)  # [S, 512]
    return attn @ wo  # row-parallel partial [S, D]


def _run_device(shards, cos, sin, group_psum=False):
    """Execute the 8 shards on the 8 NeuronCores via jax pmap (axon PJRT).

    group_psum=True reduces the row-parallel partials on device (psum within
    each batch's 4-core group) so only 2 result shards cross the host link.
    """
    import jax
    import jax.numpy as jnp

    devs = jax.devices()
    if len(devs) < N_CORES:
        raise RuntimeError(f"need {N_CORES} devices, have {len(devs)}")

    cos_j = jnp.asarray(cos)
    sin_j = jnp.asarray(sin)
    scale = 1.0 / math.sqrt(D_HEAD)

    def shard_fn(x, wq, wk, wv, wo, qw, kw):
        q = (x @ wq).reshape(S, GROUPS, D_HEAD).transpose(1, 0, 2)
        k = x @ wk
        v = x @ wv

        def rms(t, w):
            var = jnp.mean(t * t, axis=-1, keepdims=True)
            return t * jax.lax.rsqrt(var + EPS) * w

        q = rms(q, qw)
        k = rms(k, kw)

        def rope(t):
            half = D_HEAD // 2
            rot = jnp.concatenate([-t[..., half:], t[..., :half]], axis=-1)
            return t * cos_j + rot * sin_j

        q = rope(q)
        k = rope(k[None])[0]

        s = jnp.einsum("hqd,kd->hqk", q, k) * scale
        causal = jnp.tril(jnp.ones((S, S), bool))
        s = jnp.where(causal[None], s, jnp.finfo(s.dtype).min)
        p = jax.nn.softmax(s, axis=-1)
        o = jnp.einsum("hqk,kd->hqd", p, v)
        attn = o.transpose(1, 0, 2).reshape(S, GROUPS * D_HEAD)
        part = attn @ wo
        if group_psum:
            part = jax.lax.psum(
                part, axis_name="c",
                axis_index_groups=[list(range(b * GROUPS, (b + 1) * GROUPS))
                                   for b in range(B)])
        return part

    pm = jax.pmap(shard_fn, axis_name="c", devices=devs[:N_CORES])
    args = []
    for key in ("x", "wq", "wk", "wv", "wo", "qw", "kw"):
        args.append(np.stack([sh[key] for sh in shards]))
    out = pm(*args)  # [8, S, D]
    if group_psum:
        # every core in a group holds the reduced result; fetch one per batch
        return np.stack([np.asarray(out[b * GROUPS]) for b in range(B)])
    return np.asarray(out)


def kernel(x, Wq, Wk, Wv, Wo, q_norm_w, k_norm_w):
    x = np.asarray(x, np.float32)
    Wq = np.asarray(Wq, np.float32)
    Wk = np.asarray(Wk, np.float32)
    Wv = np.asarray(Wv, np.float32)
    Wo = np.asarray(Wo, np.float32)
    q_norm_w = np.asarray(q_norm_w, np.float32)
    k_norm_w = np.asarray(k_norm_w, np.float32)

    cos, sin = _rope_tables()
    shards = _shard_inputs(x, Wq, Wk, Wv, Wo, q_norm_w, k_norm_w)

    try:
        partials = _run_device(shards, cos, sin)  # [8, S, D]
    except Exception:
        partials = np.stack([
            _gqa_shard_np(sh["x"], sh["wq"], sh["wk"], sh["wv"], sh["wo"],
                          sh["qw"], sh["kw"], cos, sin)
            for sh in shards
        ])

    # row-parallel reduce over the 4 kv-groups of each batch
    out = np.empty((B, S, D), np.float32)
    for b in range(B):
        out[b] = partials[b * GROUPS:(b + 1) * GROUPS].sum(axis=0)
    return out

